# revision 2
# baseline (speedup 1.0000x reference)
"""DistMaps kernel v4 for Trainium2 (Bass), SPMD over 8 NeuronCores.

v3 over v2:
- input DMA without single_packet (v2's single_packet serialized 88KB through
  one SDMA engine: ~2.5us to data-ready).
- 6 single-bank PSUM slots for the 8 main matmuls (plus 2 ps1 banks), so
  thresholds are contiguous [128,480] group-level ops writing compact u8 and
  only the last two matmul groups wait on the earliest two thresholds.
- each batch's output is one contiguous [128, 2400B] u8 store (2 stores total).
- indicators: two double-width DVE tensor_tensor ops (z-groups 01 and 23);
  ACT spends its whole budget on thresholds.
"""

import numpy as np

B = 2
D, H, W = 96, 160, 160
P = 24
J = 10
NCORES = 8
DLOC = D // NCORES   # 12
ZG = 3               # z-slices per matmul group
NZG = DLOC // ZG     # 4
NW = ZG * W          # 480
R2 = 25.0
NWARM = 9

_prog_cache = {}


def _build_program():
    import concourse.mybir as mybir
    from concourse import bacc

    f32 = mybir.dt.float32
    bf16 = mybir.dt.bfloat16
    u8 = mybir.dt.uint8
    op = mybir.AluOpType
    Act = mybir.ActivationFunctionType

    nc = bacc.Bacc(trn_type="TRN2")

    tab_d = nc.dram_tensor("tab", [128, W + DLOC], f32, kind="ExternalInput")
    oh_d = nc.dram_tensor("oh", [128, B * W], bf16, kind="ExternalInput")
    # outall[b, :, 0:1920] = y<128 rows, z-major (12,160); [1920:2400) = ps1
    # rows (partition = 32*zg+yo -> y=128+yo), cols (zi,x)
    outall_d = nc.dram_tensor("outall", [B, 128, 2400], u8, kind="ExternalOutput")

    s_in = nc.alloc_semaphore("s_in")
    s_oh = nc.alloc_semaphore("s_oh")
    s_dve = nc.alloc_semaphore("s_dve")
    s_act = nc.alloc_semaphore("s_act")
    s_pe = nc.alloc_semaphore("s_pe")
    s_st = nc.alloc_semaphore("s_st")
    sems = (s_in, s_oh, s_dve, s_act, s_pe, s_st)
    sem_nums = [s.num for s in sems]
    assert max(sem_nums) - min(sem_nums) + 1 == len(sem_nums), sem_nums
    sem_range = range(min(sem_nums), max(sem_nums) + 1)

    c = {"dve": 0, "act": 0, "pe": 0, "st": 0}

    tab = nc.alloc_sbuf_tensor("tab_sb", [128, W + DLOC], f32)
    ohb = nc.alloc_sbuf_tensor("ohb", [128, B * W], bf16)
    rhsbig = nc.alloc_sbuf_tensor("rhsbig", [128, NZG * NW], bf16)
    rhs_t = [rhsbig[:, g * NW : (g + 1) * NW] for g in range(NZG)]
    obbig = nc.alloc_sbuf_tensor("obbig", [128, B * 2400], u8)
    warm_a = nc.alloc_sbuf_tensor("warm_a", [128, 512], bf16)
    actscratch = nc.alloc_sbuf_tensor("actscratch", [1, 4], f32)

    # 6 single-bank main slots + 2 ps1 banks = 8 PSUM banks
    ps0 = [nc.alloc_psum_tensor(f"ps0_{i}", [128, 512], f32) for i in range(6)]
    ps1 = [nc.alloc_psum_tensor(f"ps1_{b}", [128, 512], f32) for b in range(B)]
    warm_ps = ps1[1]

    # ---- input loads on the two HWDGE rings ----
    nc.gpsimd.dma_start(out=tab[:, :], in_=tab_d[:, :]).then_inc(s_in, 16)
    nc.scalar.dma_start(out=ohb[:, :], in_=oh_d[:, :]).then_inc(s_oh, 16)
    nc.scalar.activation(out=actscratch[0:1, :], in_=actscratch[0:1, :], func=Act.Sign)

    # ---- PE HAM warm-up on garbage SBUF ----
    for _ in range(NWARM):
        nc.tensor.matmul(
            out=warm_ps[:, 0:NW], lhsT=warm_a[:, 0:128], rhs=warm_a[:, 0:NW],
            start=True, stop=True,
        )

    # ---- DVE: indicators as two double-width ops (zg 0,1 then zg 2,3) ----
    rhs_ready = {}

    def emit_ind(zg0, nzg):
        # one op covering zg0 .. zg0+nzg-1
        tcols = tab[:, W + zg0 * ZG : W + (zg0 + nzg) * ZG]
        nc.vector.tensor_tensor(
            out=rhsbig[:, zg0 * NW : (zg0 + nzg) * NW].rearrange(
                "p (z x) -> p z x", x=W
            ),
            in0=tab[:, 0:W].rearrange("p x -> p () x").to_broadcast(
                [128, nzg * ZG, W]
            ),
            in1=tcols.rearrange("p z -> p z ()").to_broadcast([128, nzg * ZG, W]),
            op=op.is_le,
        ).then_inc(s_dve, 1)
        c["dve"] += 1
        v = ("dve", c["dve"])
        for g in range(zg0, zg0 + nzg):
            rhs_ready[g] = v

    nc.vector.wait_ge(s_in, 16)
    emit_ind(0, 1)
    emit_ind(1, 1)
    emit_ind(2, 2)

    def sem_of(tag):
        return {"dve": s_dve, "act": s_act}[tag]

    # ---- PE ----
    nc.tensor.wait_ge(s_oh, 16)
    mm_cnt = {}
    thr_done = {}

    def emit_main_mm(b, zg, slot, extra_wait=None):
        E, v = rhs_ready[zg]
        nc.tensor.wait_ge(sem_of(E), v)
        if extra_wait is not None:
            TE, tv = thr_done[extra_wait]
            nc.tensor.wait_ge(sem_of(TE), tv)
        nc.tensor.matmul(
            out=ps0[slot][:, 0:NW],
            lhsT=ohb[:, b * W : b * W + 128],
            rhs=rhs_t[zg][:, :],
            start=True, stop=True,
        ).then_inc(s_pe, 1)
        c["pe"] += 1
        mm_cnt[(b, zg)] = c["pe"]

    def emit_ps1_mms(b):
        for zg in range(NZG):
            ins = nc.tensor.matmul(
                out=ps1[b][32 * zg : 32 * zg + 32, 0:NW],
                lhsT=ohb[:, b * W + 128 : b * W + 160],
                rhs=rhs_t[zg][:, :],
                start=True, stop=True,
                tile_position=(0, 32 * zg),
            )
        ins.then_inc(s_pe, 1)
        c["pe"] += 1
        mm_cnt[("ps1", b)] = c["pe"]

    # ---- thresholds (contiguous [128,480] PSUM -> compact u8) ----
    def emit_thr(key, src, dst, E):
        need = mm_cnt[key]
        if E == "dve":
            nc.vector.wait_ge(s_pe, need)
            nc.vector.tensor_scalar(
                out=dst, in0=src, scalar1=0.0, scalar2=None, op0=op.is_gt
            ).then_inc(s_dve, 1)
            c["dve"] += 1
            thr_done[key] = ("dve", c["dve"])
        else:
            nc.scalar.wait_ge(s_pe, need)
            nc.scalar.activation(out=dst, in_=src, func=Act.Sign).then_inc(s_act, 1)
            c["act"] += 1
            thr_done[key] = ("act", c["act"])

    def thr_main(b, zg, slot, E):
        emit_thr(
            (b, zg),
            ps0[slot][:, 0:NW],
            obbig[:, b * 2400 + zg * NW : b * 2400 + (zg + 1) * NW],
            E,
        )

    def thr_ps1(b, E):
        emit_thr(
            ("ps1", b),
            ps1[b][:, 0:NW],
            obbig[:, b * 2400 + 1920 : b * 2400 + 2400],
            E,
        )

    # mm order / slots
    emit_main_mm(0, 0, 0)                          # 1
    emit_main_mm(0, 1, 1)                          # 2
    emit_main_mm(1, 0, 2)                          # 3
    emit_main_mm(1, 1, 3)                          # 4
    emit_main_mm(0, 2, 4)                          # 5
    emit_main_mm(0, 3, 5)                          # 6
    emit_ps1_mms(0)                                # 7

    emit_ps1_mms(1)                                # 8
    # DVE thresholds: b0g0 first (frees slot 0)
    thr_main(0, 0, 0, "dve")
    # ACT thresholds start with b0g1 (frees slot 1)
    thr_main(0, 1, 1, "act")
    # late mms reuse slots 0,1 after those thresholds
    emit_main_mm(1, 2, 0, extra_wait=(0, 0))       # 9
    emit_main_mm(1, 3, 1, extra_wait=(0, 1))       # 10

    thr_main(1, 0, 2, "dve")
    thr_main(1, 1, 3, "act")
    thr_main(0, 2, 4, "act")
    thr_ps1(0, "dve")
    thr_main(0, 3, 5, "act")
    thr_main(1, 2, 0, "dve")
    thr_main(1, 3, 1, "act")
    thr_ps1(1, "dve")

    # ---- stores: one contiguous DMA per batch ----
    def store_batch(b, ring):
        eng = nc.sync if ring == "sync" else nc.scalar
        keys = [(b, g) for g in range(NZG)] + [("ps1", b)]
        dmax = max((v for k in keys for (E, v) in [thr_done[k]] if E == "dve"),
                   default=None)
        amax = max((v for k in keys for (E, v) in [thr_done[k]] if E == "act"),
                   default=None)
        if dmax is not None:
            eng.wait_ge(s_dve, dmax)
        if amax is not None:
            eng.wait_ge(s_act, amax)
        eng.dma_start(
            out=outall_d[b, :, :], in_=obbig[:, b * 2400 : (b + 1) * 2400]
        ).then_inc(s_st, 16)

    store_batch(0, "sync")
    store_batch(1, "scalar")

    # no explicit store-completion wait or sem reset: the walrus epilogue
    # (GroupResetSemaphores) drains the DMA queues and clears every
    # semaphore, overlapping the store drain with the fixed epilogue sweep.
    nc.finalize()
    return nc


def _build_in_maps(coords: np.ndarray):
    import ml_dtypes

    coords = np.ascontiguousarray(coords, dtype=np.float32)
    assert coords.shape == (B * P, 3)
    f32 = np.float32

    xg = np.arange(W, dtype=f32)
    jcol = np.arange(-4, 6, dtype=f32)

    in_maps = []
    max_rows = 0
    for core in range(NCORES):
        z0 = core * DLOC
        zs = np.arange(z0, z0 + DLOC, dtype=f32)

        dx2 = np.full((128, W), 1.0e9, dtype=f32)
        tts = np.full((128, DLOC), -1.0e9, dtype=f32)
        oh = np.zeros((128, B * W), dtype=f32)

        r = 0
        for b in range(B):
            cb = coords[b * P : (b + 1) * P]
            for (pz, py, px) in cb:
                pz, py, px = f32(pz), f32(py), f32(px)
                dzt = (zs - pz).astype(f32)
                dzt = (dzt * dzt).astype(f32)
                yfl = f32(np.floor(py))
                ypr = (yfl + jcol).astype(f32)
                dyj = (ypr - py).astype(f32)
                dy2 = (dyj * dyj).astype(f32)
                cc = (f32(R2) - dy2).astype(f32)
                tloc = (cc[:, None] - dzt[None, :]).astype(f32)  # [J, DLOC]
                dxr = (xg - px).astype(f32)
                dxr2 = (dxr * dxr).astype(f32)
                for j in range(J):
                    y = ypr[j]
                    if y < 0 or y > H - 1:
                        continue
                    if tloc[j].max() <= 0.0:
                        continue
                    if r >= 128:
                        raise RuntimeError("kernel4: >128 active rows on a core")
                    dx2[r] = dxr2
                    tts[r] = tloc[j]
                    oh[r, b * W + int(y)] = 1.0
                    r += 1
        max_rows = max(max_rows, r)

        tab = np.concatenate([dx2, tts], axis=1)
        in_maps.append(
            {
                "tab": np.ascontiguousarray(tab, dtype=np.float32),
                "oh": np.ascontiguousarray(oh).astype(ml_dtypes.bfloat16),
            }
        )
    return max_rows, in_maps


def _get_program(npts=None):
    if "v3" not in _prog_cache:
        _prog_cache["v3"] = _build_program()
    return _prog_cache["v3"]


def kernel(x: np.ndarray, coords: np.ndarray) -> np.ndarray:
    from concourse.bass_utils import run_bass_kernel_spmd

    assert x.shape == (B, 4, D, H, W)
    _, in_maps = _build_in_maps(coords)
    nc = _get_program()
    res = run_bass_kernel_spmd(nc, in_maps, list(range(NCORES)))

    full = np.empty((B, 1, D, H, W), dtype=np.float32)
    for core in range(NCORES):
        zsl = slice(core * DLOC, (core + 1) * DLOC)
        oa = res.results[core]["outall"]  # [B, 128, 2400] u8
        om = oa[:, :, 0:1920].reshape(B, 128, DLOC, W).transpose(0, 2, 1, 3)
        full[:, 0, zsl, 0:128, :] = om
        o1 = oa[:, :, 1920:2400].reshape(B, NZG, 32, ZG, W)
        o1 = o1.transpose(0, 1, 3, 2, 4).reshape(B, DLOC, 32, W)
        full[:, 0, zsl, 128:160, :] = o1
    return full


# revision 3
# speedup vs baseline: 1.0365x; 1.0365x over previous
"""DistMaps kernel v8 (safe tail, no dummy DMAs) for Trainium2 (Bass), SPMD over 8 NeuronCores.

v3 over v2:
- input DMA without single_packet (v2's single_packet serialized 88KB through
  one SDMA engine: ~2.5us to data-ready).
- 6 single-bank PSUM slots for the 8 main matmuls (plus 2 ps1 banks), so
  thresholds are contiguous [128,480] group-level ops writing compact u8 and
  only the last two matmul groups wait on the earliest two thresholds.
- each batch's output is one contiguous [128, 2400B] u8 store (2 stores total).
- indicators: two double-width DVE tensor_tensor ops (z-groups 01 and 23);
  ACT spends its whole budget on thresholds.
"""

import numpy as np

B = 2
D, H, W = 96, 160, 160
P = 24
J = 10
NCORES = 8
DLOC = D // NCORES   # 12
ZG = 3               # z-slices per matmul group
NZG = DLOC // ZG     # 4
NW = ZG * W          # 480
R2 = 25.0
NWARM = 9

_prog_cache = {}


def _build_program():
    import concourse.mybir as mybir
    from concourse import bacc

    f32 = mybir.dt.float32
    bf16 = mybir.dt.bfloat16
    u8 = mybir.dt.uint8
    op = mybir.AluOpType
    Act = mybir.ActivationFunctionType

    nc = bacc.Bacc(trn_type="TRN2")

    tab_d = nc.dram_tensor("tab", [128, W + DLOC], f32, kind="ExternalInput")
    oh_d = nc.dram_tensor("oh", [128, B * W], bf16, kind="ExternalInput")
    # outall[b, :, 0:1920] = y<128 rows, z-major (12,160); [1920:2400) = ps1
    # rows (partition = 32*zg+yo -> y=128+yo), cols (zi,x)
    outall_d = nc.dram_tensor("outall", [B, 128, 2400], u8, kind="ExternalOutput")

    s_in = nc.alloc_semaphore("s_in")
    s_oh = nc.alloc_semaphore("s_oh")
    s_dve = nc.alloc_semaphore("s_dve")
    s_act = nc.alloc_semaphore("s_act")
    s_pe = nc.alloc_semaphore("s_pe")
    s_st = nc.alloc_semaphore("s_st")
    sems = (s_in, s_oh, s_dve, s_act, s_pe, s_st)
    sem_nums = [s.num for s in sems]
    assert max(sem_nums) - min(sem_nums) + 1 == len(sem_nums), sem_nums
    sem_range = range(min(sem_nums), max(sem_nums) + 1)

    c = {"dve": 0, "act": 0, "pe": 0, "st": 0}

    tab = nc.alloc_sbuf_tensor("tab_sb", [128, W + DLOC], f32)
    ohb = nc.alloc_sbuf_tensor("ohb", [128, B * W], bf16)
    rhsbig = nc.alloc_sbuf_tensor("rhsbig", [128, NZG * NW], bf16)
    rhs_t = [rhsbig[:, g * NW : (g + 1) * NW] for g in range(NZG)]
    obbig = nc.alloc_sbuf_tensor("obbig", [128, B * 2400], u8)
    warm_a = nc.alloc_sbuf_tensor("warm_a", [128, 512], bf16)
    actscratch = nc.alloc_sbuf_tensor("actscratch", [1, 4], f32)

    # 6 single-bank main slots + 2 ps1 banks = 8 PSUM banks
    ps0 = [nc.alloc_psum_tensor(f"ps0_{i}", [128, 512], f32) for i in range(6)]
    ps1 = [nc.alloc_psum_tensor(f"ps1_{b}", [128, 512], f32) for b in range(B)]
    warm_ps = ps1[1]

    # ---- input loads (SWDGE for tab: faster completion path) ----
    nc.gpsimd.dma_start(out=tab[:, :], in_=tab_d[:, :]).then_inc(s_in, 16)
    nc.scalar.dma_start(out=ohb[:, :], in_=oh_d[:, :]).then_inc(s_oh, 16)
    nc.scalar.activation(out=actscratch[0:1, :], in_=actscratch[0:1, :], func=Act.Sign)

    # ---- PE HAM warm-up on garbage SBUF ----
    for _ in range(NWARM):
        nc.tensor.matmul(
            out=warm_ps[:, 0:NW], lhsT=warm_a[:, 0:128], rhs=warm_a[:, 0:NW],
            start=True, stop=True,
        )

    # ---- DVE: indicators as two double-width ops (zg 0,1 then zg 2,3) ----
    rhs_ready = {}

    def emit_ind(zg0, nzg):
        # one op covering zg0 .. zg0+nzg-1
        tcols = tab[:, W + zg0 * ZG : W + (zg0 + nzg) * ZG]
        nc.vector.tensor_tensor(
            out=rhsbig[:, zg0 * NW : (zg0 + nzg) * NW].rearrange(
                "p (z x) -> p z x", x=W
            ),
            in0=tab[:, 0:W].rearrange("p x -> p () x").to_broadcast(
                [128, nzg * ZG, W]
            ),
            in1=tcols.rearrange("p z -> p z ()").to_broadcast([128, nzg * ZG, W]),
            op=op.is_le,
        ).then_inc(s_dve, 1)
        c["dve"] += 1
        v = ("dve", c["dve"])
        for g in range(zg0, zg0 + nzg):
            rhs_ready[g] = v

    nc.vector.wait_ge(s_in, 16)
    emit_ind(0, 1)
    emit_ind(1, 1)
    emit_ind(2, 2)

    def sem_of(tag):
        return {"dve": s_dve, "act": s_act}[tag]

    # ---- PE ----
    nc.tensor.wait_ge(s_oh, 16)
    mm_cnt = {}
    thr_done = {}

    def emit_main_mm(b, zg, slot, extra_wait=None):
        E, v = rhs_ready[zg]
        nc.tensor.wait_ge(sem_of(E), v)
        if extra_wait is not None:
            TE, tv = thr_done[extra_wait]
            nc.tensor.wait_ge(sem_of(TE), tv)
        nc.tensor.matmul(
            out=ps0[slot][:, 0:NW],
            lhsT=ohb[:, b * W : b * W + 128],
            rhs=rhs_t[zg][:, :],
            start=True, stop=True,
        ).then_inc(s_pe, 1)
        c["pe"] += 1
        mm_cnt[(b, zg)] = c["pe"]

    def emit_ps1_mms(b):
        for zg in range(NZG):
            ins = nc.tensor.matmul(
                out=ps1[b][32 * zg : 32 * zg + 32, 0:NW],
                lhsT=ohb[:, b * W + 128 : b * W + 160],
                rhs=rhs_t[zg][:, :],
                start=True, stop=True,
                tile_position=(0, 32 * zg),
            )
        ins.then_inc(s_pe, 1)
        c["pe"] += 1
        mm_cnt[("ps1", b)] = c["pe"]

    # ---- thresholds (contiguous [128,480] PSUM -> compact u8) ----
    def emit_thr(key, src, dst, E):
        need = mm_cnt[key]
        if E == "dve":
            nc.vector.wait_ge(s_pe, need)
            nc.vector.tensor_scalar(
                out=dst, in0=src, scalar1=0.0, scalar2=None, op0=op.is_gt
            ).then_inc(s_dve, 1)
            c["dve"] += 1
            thr_done[key] = ("dve", c["dve"])
        else:
            nc.scalar.wait_ge(s_pe, need)
            nc.scalar.activation(out=dst, in_=src, func=Act.Sign).then_inc(s_act, 1)
            c["act"] += 1
            thr_done[key] = ("act", c["act"])

    def thr_main(b, zg, slot, E):
        emit_thr(
            (b, zg),
            ps0[slot][:, 0:NW],
            obbig[:, b * 2400 + zg * NW : b * 2400 + (zg + 1) * NW],
            E,
        )

    def thr_ps1(b, E):
        emit_thr(
            ("ps1", b),
            ps1[b][:, 0:NW],
            obbig[:, b * 2400 + 1920 : b * 2400 + 2400],
            E,
        )

    # mm order / slots
    emit_main_mm(0, 0, 0)                          # 1
    emit_main_mm(0, 1, 1)                          # 2
    emit_main_mm(1, 0, 2)                          # 3
    emit_main_mm(1, 1, 3)                          # 4
    emit_main_mm(0, 2, 4)                          # 5
    emit_main_mm(0, 3, 5)                          # 6
    emit_ps1_mms(0)                                # 7

    emit_ps1_mms(1)                                # 8
    # DVE thresholds: b0g0 first (frees slot 0)
    thr_main(0, 0, 0, "dve")
    # ACT thresholds start with b0g1 (frees slot 1)
    thr_main(0, 1, 1, "act")
    # late mms reuse slots 0,1 after those thresholds
    emit_main_mm(1, 2, 0, extra_wait=(0, 0))       # 9
    emit_main_mm(1, 3, 1, extra_wait=(0, 1))       # 10

    thr_main(1, 0, 2, "dve")
    thr_main(1, 1, 3, "act")
    thr_main(0, 2, 4, "act")
    thr_ps1(0, "dve")
    thr_main(0, 3, 5, "act")
    thr_main(1, 2, 0, "dve")
    thr_main(1, 3, 1, "act")
    thr_ps1(1, "dve")

    # ---- stores ----
    def store_cols(b, lo, hi, keys, ring):
        eng = nc.sync if ring == "sync" else nc.scalar
        dmax = max((v for k in keys for (E, v) in [thr_done[k]] if E == "dve"),
                   default=None)
        amax = max((v for k in keys for (E, v) in [thr_done[k]] if E == "act"),
                   default=None)
        if dmax is not None:
            eng.wait_ge(s_dve, dmax)
        if amax is not None:
            eng.wait_ge(s_act, amax)
        eng.dma_start(
            out=outall_d[b, :, lo:hi], in_=obbig[:, b * 2400 + lo : b * 2400 + hi]
        ).then_inc(s_st, 16)

    bkeys = lambda b: [(b, g) for g in range(NZG)]
    store_cols(0, 0, 2400, bkeys(0) + [("ps1", 0)], "sync")
    store_cols(1, 0, 1920, bkeys(1), "scalar")
    store_cols(1, 1920, 2400, [("ps1", 1)], "sync")

    # ---- GPSIMD: wait for stores (incl. the 2 warm-up dummies), reset ----
    nc.gpsimd.wait_ge(s_st, 3 * 16)
    nc.gpsimd.dma_reset(sem_range)
    nc.gpsimd.sem_clear(sem_range)

    nc.finalize()
    return nc


def _build_in_maps(coords: np.ndarray):
    import ml_dtypes

    coords = np.ascontiguousarray(coords, dtype=np.float32)
    assert coords.shape == (B * P, 3)
    f32 = np.float32

    xg = np.arange(W, dtype=f32)
    jcol = np.arange(-4, 6, dtype=f32)

    in_maps = []
    max_rows = 0
    for core in range(NCORES):
        z0 = core * DLOC
        zs = np.arange(z0, z0 + DLOC, dtype=f32)

        dx2 = np.full((128, W), 1.0e9, dtype=f32)
        tts = np.full((128, DLOC), -1.0e9, dtype=f32)
        oh = np.zeros((128, B * W), dtype=f32)

        r = 0
        for b in range(B):
            cb = coords[b * P : (b + 1) * P]
            for (pz, py, px) in cb:
                pz, py, px = f32(pz), f32(py), f32(px)
                dzt = (zs - pz).astype(f32)
                dzt = (dzt * dzt).astype(f32)
                yfl = f32(np.floor(py))
                ypr = (yfl + jcol).astype(f32)
                dyj = (ypr - py).astype(f32)
                dy2 = (dyj * dyj).astype(f32)
                cc = (f32(R2) - dy2).astype(f32)
                tloc = (cc[:, None] - dzt[None, :]).astype(f32)  # [J, DLOC]
                dxr = (xg - px).astype(f32)
                dxr2 = (dxr * dxr).astype(f32)
                for j in range(J):
                    y = ypr[j]
                    if y < 0 or y > H - 1:
                        continue
                    if tloc[j].max() <= 0.0:
                        continue
                    if r >= 128:
                        raise RuntimeError("kernel4: >128 active rows on a core")
                    dx2[r] = dxr2
                    tts[r] = tloc[j]
                    oh[r, b * W + int(y)] = 1.0
                    r += 1
        max_rows = max(max_rows, r)

        tab = np.concatenate([dx2, tts], axis=1)
        in_maps.append(
            {
                "tab": np.ascontiguousarray(tab, dtype=np.float32),
                "oh": np.ascontiguousarray(oh).astype(ml_dtypes.bfloat16),
            }
        )
    return max_rows, in_maps


def _get_program(npts=None):
    if "v3" not in _prog_cache:
        _prog_cache["v3"] = _build_program()
    return _prog_cache["v3"]


def kernel(x: np.ndarray, coords: np.ndarray) -> np.ndarray:
    from concourse.bass_utils import run_bass_kernel_spmd

    assert x.shape == (B, 4, D, H, W)
    _, in_maps = _build_in_maps(coords)
    nc = _get_program()
    res = run_bass_kernel_spmd(nc, in_maps, list(range(NCORES)))

    full = np.empty((B, 1, D, H, W), dtype=np.float32)
    for core in range(NCORES):
        zsl = slice(core * DLOC, (core + 1) * DLOC)
        oa = res.results[core]["outall"]  # [B, 128, 2400] u8
        om = oa[:, :, 0:1920].reshape(B, 128, DLOC, W).transpose(0, 2, 1, 3)
        full[:, 0, zsl, 0:128, :] = om
        o1 = oa[:, :, 1920:2400].reshape(B, NZG, 32, ZG, W)
        o1 = o1.transpose(0, 1, 3, 2, 4).reshape(B, DLOC, 32, W)
        full[:, 0, zsl, 128:160, :] = o1
    return full


# revision 4
# speedup vs baseline: 1.0699x; 1.0322x over previous
"""DistMaps kernel v9 (safe tail, walrus-cleared sems) for Trainium2 (Bass), SPMD over 8 NeuronCores.

v3 over v2:
- input DMA without single_packet (v2's single_packet serialized 88KB through
  one SDMA engine: ~2.5us to data-ready).
- 6 single-bank PSUM slots for the 8 main matmuls (plus 2 ps1 banks), so
  thresholds are contiguous [128,480] group-level ops writing compact u8 and
  only the last two matmul groups wait on the earliest two thresholds.
- each batch's output is one contiguous [128, 2400B] u8 store (2 stores total).
- indicators: two double-width DVE tensor_tensor ops (z-groups 01 and 23);
  ACT spends its whole budget on thresholds.
"""

import numpy as np

B = 2
D, H, W = 96, 160, 160
P = 24
J = 10
NCORES = 8
DLOC = D // NCORES   # 12
ZG = 3               # z-slices per matmul group
NZG = DLOC // ZG     # 4
NW = ZG * W          # 480
R2 = 25.0
NWARM = 9

_prog_cache = {}


def _build_program():
    import concourse.mybir as mybir
    from concourse import bacc

    f32 = mybir.dt.float32
    bf16 = mybir.dt.bfloat16
    u8 = mybir.dt.uint8
    op = mybir.AluOpType
    Act = mybir.ActivationFunctionType

    nc = bacc.Bacc(trn_type="TRN2")

    tab_d = nc.dram_tensor("tab", [128, W + DLOC], f32, kind="ExternalInput")
    oh_d = nc.dram_tensor("oh", [128, B * W], bf16, kind="ExternalInput")
    # outall[b, :, 0:1920] = y<128 rows, z-major (12,160); [1920:2400) = ps1
    # rows (partition = 32*zg+yo -> y=128+yo), cols (zi,x)
    outall_d = nc.dram_tensor("outall", [B, 128, 2400], u8, kind="ExternalOutput")

    s_in = nc.alloc_semaphore("s_in")
    s_oh = nc.alloc_semaphore("s_oh")
    s_dve = nc.alloc_semaphore("s_dve")
    s_act = nc.alloc_semaphore("s_act")
    s_pe = nc.alloc_semaphore("s_pe")
    s_st = nc.alloc_semaphore("s_st")
    sems = (s_in, s_oh, s_dve, s_act, s_pe, s_st)
    sem_nums = [s.num for s in sems]
    assert max(sem_nums) - min(sem_nums) + 1 == len(sem_nums), sem_nums
    sem_range = range(min(sem_nums), max(sem_nums) + 1)

    c = {"dve": 0, "act": 0, "pe": 0, "st": 0}

    tab = nc.alloc_sbuf_tensor("tab_sb", [128, W + DLOC], f32)
    ohb = nc.alloc_sbuf_tensor("ohb", [128, B * W], bf16)
    rhsbig = nc.alloc_sbuf_tensor("rhsbig", [128, NZG * NW], bf16)
    rhs_t = [rhsbig[:, g * NW : (g + 1) * NW] for g in range(NZG)]
    obbig = nc.alloc_sbuf_tensor("obbig", [128, B * 2400], u8)
    warm_a = nc.alloc_sbuf_tensor("warm_a", [128, 512], bf16)
    actscratch = nc.alloc_sbuf_tensor("actscratch", [1, 4], f32)

    # 6 single-bank main slots + 2 ps1 banks = 8 PSUM banks
    ps0 = [nc.alloc_psum_tensor(f"ps0_{i}", [128, 512], f32) for i in range(6)]
    ps1 = [nc.alloc_psum_tensor(f"ps1_{b}", [128, 512], f32) for b in range(B)]
    warm_ps = ps1[1]

    # ---- input loads (SWDGE for tab: faster completion path) ----
    nc.gpsimd.dma_start(out=tab[:, :], in_=tab_d[:, :]).then_inc(s_in, 16)
    nc.scalar.dma_start(out=ohb[:, :], in_=oh_d[:, :]).then_inc(s_oh, 16)
    nc.scalar.activation(out=actscratch[0:1, :], in_=actscratch[0:1, :], func=Act.Sign)

    # ---- PE HAM warm-up on garbage SBUF ----
    for _ in range(NWARM):
        nc.tensor.matmul(
            out=warm_ps[:, 0:NW], lhsT=warm_a[:, 0:128], rhs=warm_a[:, 0:NW],
            start=True, stop=True,
        )

    # ---- DVE: indicators as two double-width ops (zg 0,1 then zg 2,3) ----
    rhs_ready = {}

    def emit_ind(zg0, nzg):
        # one op covering zg0 .. zg0+nzg-1
        tcols = tab[:, W + zg0 * ZG : W + (zg0 + nzg) * ZG]
        nc.vector.tensor_tensor(
            out=rhsbig[:, zg0 * NW : (zg0 + nzg) * NW].rearrange(
                "p (z x) -> p z x", x=W
            ),
            in0=tab[:, 0:W].rearrange("p x -> p () x").to_broadcast(
                [128, nzg * ZG, W]
            ),
            in1=tcols.rearrange("p z -> p z ()").to_broadcast([128, nzg * ZG, W]),
            op=op.is_le,
        ).then_inc(s_dve, 1)
        c["dve"] += 1
        v = ("dve", c["dve"])
        for g in range(zg0, zg0 + nzg):
            rhs_ready[g] = v

    nc.vector.wait_ge(s_in, 16)
    emit_ind(0, 1)
    emit_ind(1, 1)
    emit_ind(2, 2)

    def sem_of(tag):
        return {"dve": s_dve, "act": s_act}[tag]

    # ---- PE ----
    nc.tensor.wait_ge(s_oh, 16)
    mm_cnt = {}
    thr_done = {}

    def emit_main_mm(b, zg, slot, extra_wait=None):
        E, v = rhs_ready[zg]
        nc.tensor.wait_ge(sem_of(E), v)
        if extra_wait is not None:
            TE, tv = thr_done[extra_wait]
            nc.tensor.wait_ge(sem_of(TE), tv)
        nc.tensor.matmul(
            out=ps0[slot][:, 0:NW],
            lhsT=ohb[:, b * W : b * W + 128],
            rhs=rhs_t[zg][:, :],
            start=True, stop=True,
        ).then_inc(s_pe, 1)
        c["pe"] += 1
        mm_cnt[(b, zg)] = c["pe"]

    def emit_ps1_mms(b):
        for zg in range(NZG):
            ins = nc.tensor.matmul(
                out=ps1[b][32 * zg : 32 * zg + 32, 0:NW],
                lhsT=ohb[:, b * W + 128 : b * W + 160],
                rhs=rhs_t[zg][:, :],
                start=True, stop=True,
                tile_position=(0, 32 * zg),
            )
        ins.then_inc(s_pe, 1)
        c["pe"] += 1
        mm_cnt[("ps1", b)] = c["pe"]

    # ---- thresholds (contiguous [128,480] PSUM -> compact u8) ----
    def emit_thr(key, src, dst, E):
        need = mm_cnt[key]
        if E == "dve":
            nc.vector.wait_ge(s_pe, need)
            nc.vector.tensor_scalar(
                out=dst, in0=src, scalar1=0.0, scalar2=None, op0=op.is_gt
            ).then_inc(s_dve, 1)
            c["dve"] += 1
            thr_done[key] = ("dve", c["dve"])
        else:
            nc.scalar.wait_ge(s_pe, need)
            nc.scalar.activation(out=dst, in_=src, func=Act.Sign).then_inc(s_act, 1)
            c["act"] += 1
            thr_done[key] = ("act", c["act"])

    def thr_main(b, zg, slot, E):
        emit_thr(
            (b, zg),
            ps0[slot][:, 0:NW],
            obbig[:, b * 2400 + zg * NW : b * 2400 + (zg + 1) * NW],
            E,
        )

    def thr_ps1(b, E):
        emit_thr(
            ("ps1", b),
            ps1[b][:, 0:NW],
            obbig[:, b * 2400 + 1920 : b * 2400 + 2400],
            E,
        )

    # mm order / slots
    emit_main_mm(0, 0, 0)                          # 1
    emit_main_mm(0, 1, 1)                          # 2
    emit_main_mm(1, 0, 2)                          # 3
    emit_main_mm(1, 1, 3)                          # 4
    emit_main_mm(0, 2, 4)                          # 5
    emit_main_mm(0, 3, 5)                          # 6
    emit_ps1_mms(0)                                # 7

    emit_ps1_mms(1)                                # 8
    # DVE thresholds: b0g0 first (frees slot 0)
    thr_main(0, 0, 0, "dve")
    # ACT thresholds start with b0g1 (frees slot 1)
    thr_main(0, 1, 1, "act")
    # late mms reuse slots 0,1 after those thresholds
    emit_main_mm(1, 2, 0, extra_wait=(0, 0))       # 9
    emit_main_mm(1, 3, 1, extra_wait=(0, 1))       # 10

    thr_main(1, 0, 2, "dve")
    thr_main(1, 1, 3, "act")
    thr_main(0, 2, 4, "act")
    thr_ps1(0, "dve")
    thr_main(0, 3, 5, "act")
    thr_main(1, 2, 0, "dve")
    thr_main(1, 3, 1, "act")
    thr_ps1(1, "dve")

    # ---- stores ----
    def store_cols(b, lo, hi, keys, ring):
        eng = nc.sync if ring == "sync" else nc.scalar
        dmax = max((v for k in keys for (E, v) in [thr_done[k]] if E == "dve"),
                   default=None)
        amax = max((v for k in keys for (E, v) in [thr_done[k]] if E == "act"),
                   default=None)
        if dmax is not None:
            eng.wait_ge(s_dve, dmax)
        if amax is not None:
            eng.wait_ge(s_act, amax)
        eng.dma_start(
            out=outall_d[b, :, lo:hi], in_=obbig[:, b * 2400 + lo : b * 2400 + hi]
        ).then_inc(s_st, 16)

    bkeys = lambda b: [(b, g) for g in range(NZG)]
    store_cols(0, 0, 2400, bkeys(0) + [("ps1", 0)], "sync")
    store_cols(1, 0, 1920, bkeys(1), "scalar")
    store_cols(1, 1920, 2400, [("ps1", 1)], "sync")

    # ---- GPSIMD: wait for store completion (quiesce before the walrus
    # epilogue resets DMA state).  Sem clearing itself is left to the walrus
    # epilogue sweep, which zeroes all 256 semaphores every execution.
    nc.gpsimd.wait_ge(s_st, 3 * 16)

    nc.finalize()
    return nc


def _build_in_maps(coords: np.ndarray):
    import ml_dtypes

    coords = np.ascontiguousarray(coords, dtype=np.float32)
    assert coords.shape == (B * P, 3)
    f32 = np.float32

    xg = np.arange(W, dtype=f32)
    jcol = np.arange(-4, 6, dtype=f32)

    in_maps = []
    max_rows = 0
    for core in range(NCORES):
        z0 = core * DLOC
        zs = np.arange(z0, z0 + DLOC, dtype=f32)

        dx2 = np.full((128, W), 1.0e9, dtype=f32)
        tts = np.full((128, DLOC), -1.0e9, dtype=f32)
        oh = np.zeros((128, B * W), dtype=f32)

        r = 0
        for b in range(B):
            cb = coords[b * P : (b + 1) * P]
            for (pz, py, px) in cb:
                pz, py, px = f32(pz), f32(py), f32(px)
                dzt = (zs - pz).astype(f32)
                dzt = (dzt * dzt).astype(f32)
                yfl = f32(np.floor(py))
                ypr = (yfl + jcol).astype(f32)
                dyj = (ypr - py).astype(f32)
                dy2 = (dyj * dyj).astype(f32)
                cc = (f32(R2) - dy2).astype(f32)
                tloc = (cc[:, None] - dzt[None, :]).astype(f32)  # [J, DLOC]
                dxr = (xg - px).astype(f32)
                dxr2 = (dxr * dxr).astype(f32)
                for j in range(J):
                    y = ypr[j]
                    if y < 0 or y > H - 1:
                        continue
                    if tloc[j].max() <= 0.0:
                        continue
                    if r >= 128:
                        raise RuntimeError("kernel4: >128 active rows on a core")
                    dx2[r] = dxr2
                    tts[r] = tloc[j]
                    oh[r, b * W + int(y)] = 1.0
                    r += 1
        max_rows = max(max_rows, r)

        tab = np.concatenate([dx2, tts], axis=1)
        in_maps.append(
            {
                "tab": np.ascontiguousarray(tab, dtype=np.float32),
                "oh": np.ascontiguousarray(oh).astype(ml_dtypes.bfloat16),
            }
        )
    return max_rows, in_maps


def _get_program(npts=None):
    if "v3" not in _prog_cache:
        _prog_cache["v3"] = _build_program()
    return _prog_cache["v3"]


def kernel(x: np.ndarray, coords: np.ndarray) -> np.ndarray:
    from concourse.bass_utils import run_bass_kernel_spmd

    assert x.shape == (B, 4, D, H, W)
    _, in_maps = _build_in_maps(coords)
    nc = _get_program()
    res = run_bass_kernel_spmd(nc, in_maps, list(range(NCORES)))

    full = np.empty((B, 1, D, H, W), dtype=np.float32)
    for core in range(NCORES):
        zsl = slice(core * DLOC, (core + 1) * DLOC)
        oa = res.results[core]["outall"]  # [B, 128, 2400] u8
        om = oa[:, :, 0:1920].reshape(B, 128, DLOC, W).transpose(0, 2, 1, 3)
        full[:, 0, zsl, 0:128, :] = om
        o1 = oa[:, :, 1920:2400].reshape(B, NZG, 32, ZG, W)
        o1 = o1.transpose(0, 1, 3, 2, 4).reshape(B, DLOC, 32, W)
        full[:, 0, zsl, 128:160, :] = o1
    return full


# revision 5
# speedup vs baseline: 1.0907x; 1.0194x over previous
"""DistMaps kernel v10 (scalar-ring input) for Trainium2 (Bass), SPMD over 8 NeuronCores.

v3 over v2:
- input DMA without single_packet (v2's single_packet serialized 88KB through
  one SDMA engine: ~2.5us to data-ready).
- 6 single-bank PSUM slots for the 8 main matmuls (plus 2 ps1 banks), so
  thresholds are contiguous [128,480] group-level ops writing compact u8 and
  only the last two matmul groups wait on the earliest two thresholds.
- each batch's output is one contiguous [128, 2400B] u8 store (2 stores total).
- indicators: two double-width DVE tensor_tensor ops (z-groups 01 and 23);
  ACT spends its whole budget on thresholds.
"""

import numpy as np

B = 2
D, H, W = 96, 160, 160
P = 24
J = 10
NCORES = 8
DLOC = D // NCORES   # 12
ZG = 3               # z-slices per matmul group
NZG = DLOC // ZG     # 4
NW = ZG * W          # 480
R2 = 25.0
NWARM = 9

_prog_cache = {}


def _build_program():
    import concourse.mybir as mybir
    from concourse import bacc

    f32 = mybir.dt.float32
    bf16 = mybir.dt.bfloat16
    u8 = mybir.dt.uint8
    op = mybir.AluOpType
    Act = mybir.ActivationFunctionType

    nc = bacc.Bacc(trn_type="TRN2")

    tab_d = nc.dram_tensor("tab", [128, W + DLOC], f32, kind="ExternalInput")
    oh_d = nc.dram_tensor("oh", [128, B * W], bf16, kind="ExternalInput")
    # outall[b, :, 0:1920] = y<128 rows, z-major (12,160); [1920:2400) = ps1
    # rows (partition = 32*zg+yo -> y=128+yo), cols (zi,x)
    outall_d = nc.dram_tensor("outall", [B, 128, 2400], u8, kind="ExternalOutput")

    s_in = nc.alloc_semaphore("s_in")
    s_oh = nc.alloc_semaphore("s_oh")
    s_dve = nc.alloc_semaphore("s_dve")
    s_act = nc.alloc_semaphore("s_act")
    s_pe = nc.alloc_semaphore("s_pe")
    s_st = nc.alloc_semaphore("s_st")
    sems = (s_in, s_oh, s_dve, s_act, s_pe, s_st)
    sem_nums = [s.num for s in sems]
    assert max(sem_nums) - min(sem_nums) + 1 == len(sem_nums), sem_nums
    sem_range = range(min(sem_nums), max(sem_nums) + 1)

    c = {"dve": 0, "act": 0, "pe": 0, "st": 0}

    tab = nc.alloc_sbuf_tensor("tab_sb", [128, W + DLOC], f32)
    ohb = nc.alloc_sbuf_tensor("ohb", [128, B * W], bf16)
    rhsbig = nc.alloc_sbuf_tensor("rhsbig", [128, NZG * NW], bf16)
    rhs_t = [rhsbig[:, g * NW : (g + 1) * NW] for g in range(NZG)]
    obbig = nc.alloc_sbuf_tensor("obbig", [128, B * 2400], u8)
    warm_a = nc.alloc_sbuf_tensor("warm_a", [128, 512], bf16)
    actscratch = nc.alloc_sbuf_tensor("actscratch", [1, 4], f32)

    # 6 single-bank main slots + 2 ps1 banks = 8 PSUM banks
    ps0 = [nc.alloc_psum_tensor(f"ps0_{i}", [128, 512], f32) for i in range(6)]
    ps1 = [nc.alloc_psum_tensor(f"ps1_{b}", [128, 512], f32) for b in range(B)]
    warm_ps = ps1[1]

    # ---- input loads: tab on the scalar ring (earliest-starting engine
    # preamble -> earliest issue, which gates the whole vector pipeline);
    # oh on the sync ring in parallel ----
    nc.scalar.dma_start(out=tab[:, :], in_=tab_d[:, :]).then_inc(s_in, 16)
    nc.sync.dma_start(out=ohb[:, :], in_=oh_d[:, :]).then_inc(s_oh, 16)
    nc.scalar.activation(out=actscratch[0:1, :], in_=actscratch[0:1, :], func=Act.Sign)

    # ---- PE HAM warm-up on garbage SBUF ----
    for _ in range(NWARM):
        nc.tensor.matmul(
            out=warm_ps[:, 0:NW], lhsT=warm_a[:, 0:128], rhs=warm_a[:, 0:NW],
            start=True, stop=True,
        )

    # ---- DVE: indicators as two double-width ops (zg 0,1 then zg 2,3) ----
    rhs_ready = {}

    def emit_ind(zg0, nzg):
        # one op covering zg0 .. zg0+nzg-1
        tcols = tab[:, W + zg0 * ZG : W + (zg0 + nzg) * ZG]
        nc.vector.tensor_tensor(
            out=rhsbig[:, zg0 * NW : (zg0 + nzg) * NW].rearrange(
                "p (z x) -> p z x", x=W
            ),
            in0=tab[:, 0:W].rearrange("p x -> p () x").to_broadcast(
                [128, nzg * ZG, W]
            ),
            in1=tcols.rearrange("p z -> p z ()").to_broadcast([128, nzg * ZG, W]),
            op=op.is_le,
        ).then_inc(s_dve, 1)
        c["dve"] += 1
        v = ("dve", c["dve"])
        for g in range(zg0, zg0 + nzg):
            rhs_ready[g] = v

    nc.vector.wait_ge(s_in, 16)
    emit_ind(0, 1)
    emit_ind(1, 1)
    emit_ind(2, 2)

    def sem_of(tag):
        return {"dve": s_dve, "act": s_act}[tag]

    # ---- PE ----
    nc.tensor.wait_ge(s_oh, 16)
    mm_cnt = {}
    thr_done = {}

    def emit_main_mm(b, zg, slot, extra_wait=None):
        E, v = rhs_ready[zg]
        nc.tensor.wait_ge(sem_of(E), v)
        if extra_wait is not None:
            TE, tv = thr_done[extra_wait]
            nc.tensor.wait_ge(sem_of(TE), tv)
        nc.tensor.matmul(
            out=ps0[slot][:, 0:NW],
            lhsT=ohb[:, b * W : b * W + 128],
            rhs=rhs_t[zg][:, :],
            start=True, stop=True,
        ).then_inc(s_pe, 1)
        c["pe"] += 1
        mm_cnt[(b, zg)] = c["pe"]

    def emit_ps1_mms(b):
        for zg in range(NZG):
            ins = nc.tensor.matmul(
                out=ps1[b][32 * zg : 32 * zg + 32, 0:NW],
                lhsT=ohb[:, b * W + 128 : b * W + 160],
                rhs=rhs_t[zg][:, :],
                start=True, stop=True,
                tile_position=(0, 32 * zg),
            )
        ins.then_inc(s_pe, 1)
        c["pe"] += 1
        mm_cnt[("ps1", b)] = c["pe"]

    # ---- thresholds (contiguous [128,480] PSUM -> compact u8) ----
    def emit_thr(key, src, dst, E):
        need = mm_cnt[key]
        if E == "dve":
            nc.vector.wait_ge(s_pe, need)
            nc.vector.tensor_scalar(
                out=dst, in0=src, scalar1=0.0, scalar2=None, op0=op.is_gt
            ).then_inc(s_dve, 1)
            c["dve"] += 1
            thr_done[key] = ("dve", c["dve"])
        else:
            nc.scalar.wait_ge(s_pe, need)
            nc.scalar.activation(out=dst, in_=src, func=Act.Sign).then_inc(s_act, 1)
            c["act"] += 1
            thr_done[key] = ("act", c["act"])

    def thr_main(b, zg, slot, E):
        emit_thr(
            (b, zg),
            ps0[slot][:, 0:NW],
            obbig[:, b * 2400 + zg * NW : b * 2400 + (zg + 1) * NW],
            E,
        )

    def thr_ps1(b, E):
        emit_thr(
            ("ps1", b),
            ps1[b][:, 0:NW],
            obbig[:, b * 2400 + 1920 : b * 2400 + 2400],
            E,
        )

    # mm order / slots
    emit_main_mm(0, 0, 0)                          # 1
    emit_main_mm(0, 1, 1)                          # 2
    emit_main_mm(1, 0, 2)                          # 3
    emit_main_mm(1, 1, 3)                          # 4
    emit_main_mm(0, 2, 4)                          # 5
    emit_main_mm(0, 3, 5)                          # 6
    emit_ps1_mms(0)                                # 7

    emit_ps1_mms(1)                                # 8
    # DVE thresholds: b0g0 first (frees slot 0)
    thr_main(0, 0, 0, "dve")
    # ACT thresholds start with b0g1 (frees slot 1)
    thr_main(0, 1, 1, "act")
    # late mms reuse slots 0,1 after those thresholds
    emit_main_mm(1, 2, 0, extra_wait=(0, 0))       # 9
    emit_main_mm(1, 3, 1, extra_wait=(0, 1))       # 10

    thr_main(1, 0, 2, "dve")
    thr_main(1, 1, 3, "act")
    thr_main(0, 2, 4, "act")
    thr_ps1(0, "dve")
    thr_main(0, 3, 5, "act")
    thr_main(1, 2, 0, "dve")
    thr_main(1, 3, 1, "act")
    thr_ps1(1, "dve")

    # ---- stores ----
    def store_cols(b, lo, hi, keys, ring):
        eng = nc.sync if ring == "sync" else nc.scalar
        dmax = max((v for k in keys for (E, v) in [thr_done[k]] if E == "dve"),
                   default=None)
        amax = max((v for k in keys for (E, v) in [thr_done[k]] if E == "act"),
                   default=None)
        if dmax is not None:
            eng.wait_ge(s_dve, dmax)
        if amax is not None:
            eng.wait_ge(s_act, amax)
        eng.dma_start(
            out=outall_d[b, :, lo:hi], in_=obbig[:, b * 2400 + lo : b * 2400 + hi]
        ).then_inc(s_st, 16)

    bkeys = lambda b: [(b, g) for g in range(NZG)]
    store_cols(0, 0, 2400, bkeys(0) + [("ps1", 0)], "sync")
    store_cols(1, 0, 1920, bkeys(1), "scalar")
    store_cols(1, 1920, 2400, [("ps1", 1)], "sync")

    # ---- GPSIMD: wait for store completion (quiesce before the walrus
    # epilogue resets DMA state).  Sem clearing itself is left to the walrus
    # epilogue sweep, which zeroes all 256 semaphores every execution.
    nc.gpsimd.wait_ge(s_st, 3 * 16)

    nc.finalize()
    return nc


def _build_in_maps(coords: np.ndarray):
    import ml_dtypes

    coords = np.ascontiguousarray(coords, dtype=np.float32)
    assert coords.shape == (B * P, 3)
    f32 = np.float32

    xg = np.arange(W, dtype=f32)
    jcol = np.arange(-4, 6, dtype=f32)

    in_maps = []
    max_rows = 0
    for core in range(NCORES):
        z0 = core * DLOC
        zs = np.arange(z0, z0 + DLOC, dtype=f32)

        dx2 = np.full((128, W), 1.0e9, dtype=f32)
        tts = np.full((128, DLOC), -1.0e9, dtype=f32)
        oh = np.zeros((128, B * W), dtype=f32)

        r = 0
        for b in range(B):
            cb = coords[b * P : (b + 1) * P]
            for (pz, py, px) in cb:
                pz, py, px = f32(pz), f32(py), f32(px)
                dzt = (zs - pz).astype(f32)
                dzt = (dzt * dzt).astype(f32)
                yfl = f32(np.floor(py))
                ypr = (yfl + jcol).astype(f32)
                dyj = (ypr - py).astype(f32)
                dy2 = (dyj * dyj).astype(f32)
                cc = (f32(R2) - dy2).astype(f32)
                tloc = (cc[:, None] - dzt[None, :]).astype(f32)  # [J, DLOC]
                dxr = (xg - px).astype(f32)
                dxr2 = (dxr * dxr).astype(f32)
                for j in range(J):
                    y = ypr[j]
                    if y < 0 or y > H - 1:
                        continue
                    if tloc[j].max() <= 0.0:
                        continue
                    if r >= 128:
                        raise RuntimeError("kernel4: >128 active rows on a core")
                    dx2[r] = dxr2
                    tts[r] = tloc[j]
                    oh[r, b * W + int(y)] = 1.0
                    r += 1
        max_rows = max(max_rows, r)

        tab = np.concatenate([dx2, tts], axis=1)
        in_maps.append(
            {
                "tab": np.ascontiguousarray(tab, dtype=np.float32),
                "oh": np.ascontiguousarray(oh).astype(ml_dtypes.bfloat16),
            }
        )
    return max_rows, in_maps


def _get_program(npts=None):
    if "v3" not in _prog_cache:
        _prog_cache["v3"] = _build_program()
    return _prog_cache["v3"]


def kernel(x: np.ndarray, coords: np.ndarray) -> np.ndarray:
    from concourse.bass_utils import run_bass_kernel_spmd

    assert x.shape == (B, 4, D, H, W)
    _, in_maps = _build_in_maps(coords)
    nc = _get_program()
    res = run_bass_kernel_spmd(nc, in_maps, list(range(NCORES)))

    full = np.empty((B, 1, D, H, W), dtype=np.float32)
    for core in range(NCORES):
        zsl = slice(core * DLOC, (core + 1) * DLOC)
        oa = res.results[core]["outall"]  # [B, 128, 2400] u8
        om = oa[:, :, 0:1920].reshape(B, 128, DLOC, W).transpose(0, 2, 1, 3)
        full[:, 0, zsl, 0:128, :] = om
        o1 = oa[:, :, 1920:2400].reshape(B, NZG, 32, ZG, W)
        o1 = o1.transpose(0, 1, 3, 2, 4).reshape(B, DLOC, 32, W)
        full[:, 0, zsl, 128:160, :] = o1
    return full


# revision 6
# speedup vs baseline: 1.1915x; 1.0924x over previous
"""DistMaps kernel v12 (scalar-ring input, NWARM=8) for Trainium2 (Bass), SPMD over 8 NeuronCores.

v3 over v2:
- input DMA without single_packet (v2's single_packet serialized 88KB through
  one SDMA engine: ~2.5us to data-ready).
- 6 single-bank PSUM slots for the 8 main matmuls (plus 2 ps1 banks), so
  thresholds are contiguous [128,480] group-level ops writing compact u8 and
  only the last two matmul groups wait on the earliest two thresholds.
- each batch's output is one contiguous [128, 2400B] u8 store (2 stores total).
- indicators: two double-width DVE tensor_tensor ops (z-groups 01 and 23);
  ACT spends its whole budget on thresholds.
"""

import numpy as np

B = 2
D, H, W = 96, 160, 160
P = 24
J = 10
NCORES = 8
DLOC = D // NCORES   # 12
ZG = 3               # z-slices per matmul group
NZG = DLOC // ZG     # 4
NW = ZG * W          # 480
R2 = 25.0
NWARM = 8

_prog_cache = {}


def _build_program():
    import concourse.mybir as mybir
    from concourse import bacc

    f32 = mybir.dt.float32
    bf16 = mybir.dt.bfloat16
    u8 = mybir.dt.uint8
    op = mybir.AluOpType
    Act = mybir.ActivationFunctionType

    nc = bacc.Bacc(trn_type="TRN2")

    tab_d = nc.dram_tensor("tab", [128, W + DLOC], f32, kind="ExternalInput")
    oh_d = nc.dram_tensor("oh", [128, B * W], bf16, kind="ExternalInput")
    # outall[b, :, 0:1920] = y<128 rows, z-major (12,160); [1920:2400) = ps1
    # rows (partition = 32*zg+yo -> y=128+yo), cols (zi,x)
    outall_d = nc.dram_tensor("outall", [B, 128, 2400], u8, kind="ExternalOutput")

    s_in = nc.alloc_semaphore("s_in")
    s_oh = nc.alloc_semaphore("s_oh")
    s_dve = nc.alloc_semaphore("s_dve")
    s_act = nc.alloc_semaphore("s_act")
    s_pe = nc.alloc_semaphore("s_pe")
    s_st = nc.alloc_semaphore("s_st")
    sems = (s_in, s_oh, s_dve, s_act, s_pe, s_st)
    sem_nums = [s.num for s in sems]
    assert max(sem_nums) - min(sem_nums) + 1 == len(sem_nums), sem_nums
    sem_range = range(min(sem_nums), max(sem_nums) + 1)

    c = {"dve": 0, "act": 0, "pe": 0, "st": 0}

    tab = nc.alloc_sbuf_tensor("tab_sb", [128, W + DLOC], f32)
    ohb = nc.alloc_sbuf_tensor("ohb", [128, B * W], bf16)
    rhsbig = nc.alloc_sbuf_tensor("rhsbig", [128, NZG * NW], bf16)
    rhs_t = [rhsbig[:, g * NW : (g + 1) * NW] for g in range(NZG)]
    obbig = nc.alloc_sbuf_tensor("obbig", [128, B * 2400], u8)
    warm_a = nc.alloc_sbuf_tensor("warm_a", [128, 512], bf16)
    actscratch = nc.alloc_sbuf_tensor("actscratch", [1, 4], f32)

    # 6 single-bank main slots + 2 ps1 banks = 8 PSUM banks
    ps0 = [nc.alloc_psum_tensor(f"ps0_{i}", [128, 512], f32) for i in range(6)]
    ps1 = [nc.alloc_psum_tensor(f"ps1_{b}", [128, 512], f32) for b in range(B)]
    warm_ps = ps1[1]

    # ---- input loads: tab on the scalar ring (earliest-starting engine
    # preamble -> earliest issue, which gates the whole vector pipeline);
    # oh on the sync ring in parallel ----
    nc.scalar.dma_start(out=tab[:, :], in_=tab_d[:, :]).then_inc(s_in, 16)
    nc.sync.dma_start(out=ohb[:, :], in_=oh_d[:, :]).then_inc(s_oh, 16)
    nc.scalar.activation(out=actscratch[0:1, :], in_=actscratch[0:1, :], func=Act.Sign)

    # ---- PE HAM warm-up on garbage SBUF ----
    for _ in range(NWARM):
        nc.tensor.matmul(
            out=warm_ps[:, 0:NW], lhsT=warm_a[:, 0:128], rhs=warm_a[:, 0:NW],
            start=True, stop=True,
        )

    # ---- DVE: indicators as two double-width ops (zg 0,1 then zg 2,3) ----
    rhs_ready = {}

    def emit_ind(zg0, nzg):
        # one op covering zg0 .. zg0+nzg-1
        tcols = tab[:, W + zg0 * ZG : W + (zg0 + nzg) * ZG]
        nc.vector.tensor_tensor(
            out=rhsbig[:, zg0 * NW : (zg0 + nzg) * NW].rearrange(
                "p (z x) -> p z x", x=W
            ),
            in0=tab[:, 0:W].rearrange("p x -> p () x").to_broadcast(
                [128, nzg * ZG, W]
            ),
            in1=tcols.rearrange("p z -> p z ()").to_broadcast([128, nzg * ZG, W]),
            op=op.is_le,
        ).then_inc(s_dve, 1)
        c["dve"] += 1
        v = ("dve", c["dve"])
        for g in range(zg0, zg0 + nzg):
            rhs_ready[g] = v

    nc.vector.wait_ge(s_in, 16)
    emit_ind(0, 1)
    emit_ind(1, 1)
    emit_ind(2, 2)

    def sem_of(tag):
        return {"dve": s_dve, "act": s_act}[tag]

    # ---- PE ----
    nc.tensor.wait_ge(s_oh, 16)
    mm_cnt = {}
    thr_done = {}

    def emit_main_mm(b, zg, slot, extra_wait=None):
        E, v = rhs_ready[zg]
        nc.tensor.wait_ge(sem_of(E), v)
        if extra_wait is not None:
            TE, tv = thr_done[extra_wait]
            nc.tensor.wait_ge(sem_of(TE), tv)
        nc.tensor.matmul(
            out=ps0[slot][:, 0:NW],
            lhsT=ohb[:, b * W : b * W + 128],
            rhs=rhs_t[zg][:, :],
            start=True, stop=True,
        ).then_inc(s_pe, 1)
        c["pe"] += 1
        mm_cnt[(b, zg)] = c["pe"]

    def emit_ps1_mms(b):
        for zg in range(NZG):
            ins = nc.tensor.matmul(
                out=ps1[b][32 * zg : 32 * zg + 32, 0:NW],
                lhsT=ohb[:, b * W + 128 : b * W + 160],
                rhs=rhs_t[zg][:, :],
                start=True, stop=True,
                tile_position=(0, 32 * zg),
            )
        ins.then_inc(s_pe, 1)
        c["pe"] += 1
        mm_cnt[("ps1", b)] = c["pe"]

    # ---- thresholds (contiguous [128,480] PSUM -> compact u8) ----
    def emit_thr(key, src, dst, E):
        need = mm_cnt[key]
        if E == "dve":
            nc.vector.wait_ge(s_pe, need)
            nc.vector.tensor_scalar(
                out=dst, in0=src, scalar1=0.0, scalar2=None, op0=op.is_gt
            ).then_inc(s_dve, 1)
            c["dve"] += 1
            thr_done[key] = ("dve", c["dve"])
        else:
            nc.scalar.wait_ge(s_pe, need)
            nc.scalar.activation(out=dst, in_=src, func=Act.Sign).then_inc(s_act, 1)
            c["act"] += 1
            thr_done[key] = ("act", c["act"])

    def thr_main(b, zg, slot, E):
        emit_thr(
            (b, zg),
            ps0[slot][:, 0:NW],
            obbig[:, b * 2400 + zg * NW : b * 2400 + (zg + 1) * NW],
            E,
        )

    def thr_ps1(b, E):
        emit_thr(
            ("ps1", b),
            ps1[b][:, 0:NW],
            obbig[:, b * 2400 + 1920 : b * 2400 + 2400],
            E,
        )

    # mm order / slots
    emit_main_mm(0, 0, 0)                          # 1
    emit_main_mm(0, 1, 1)                          # 2
    emit_main_mm(1, 0, 2)                          # 3
    emit_main_mm(1, 1, 3)                          # 4
    emit_main_mm(0, 2, 4)                          # 5
    emit_main_mm(0, 3, 5)                          # 6
    emit_ps1_mms(0)                                # 7

    emit_ps1_mms(1)                                # 8
    # DVE thresholds: b0g0 first (frees slot 0)
    thr_main(0, 0, 0, "dve")
    # ACT thresholds start with b0g1 (frees slot 1)
    thr_main(0, 1, 1, "act")
    # late mms reuse slots 0,1 after those thresholds
    emit_main_mm(1, 2, 0, extra_wait=(0, 0))       # 9
    emit_main_mm(1, 3, 1, extra_wait=(0, 1))       # 10

    thr_main(1, 0, 2, "dve")
    thr_main(1, 1, 3, "act")
    thr_main(0, 2, 4, "act")
    thr_ps1(0, "dve")
    thr_main(0, 3, 5, "act")
    thr_main(1, 2, 0, "dve")
    thr_main(1, 3, 1, "act")
    thr_ps1(1, "dve")

    # ---- stores ----
    def store_cols(b, lo, hi, keys, ring):
        eng = nc.sync if ring == "sync" else nc.scalar
        dmax = max((v for k in keys for (E, v) in [thr_done[k]] if E == "dve"),
                   default=None)
        amax = max((v for k in keys for (E, v) in [thr_done[k]] if E == "act"),
                   default=None)
        if dmax is not None:
            eng.wait_ge(s_dve, dmax)
        if amax is not None:
            eng.wait_ge(s_act, amax)
        eng.dma_start(
            out=outall_d[b, :, lo:hi], in_=obbig[:, b * 2400 + lo : b * 2400 + hi]
        ).then_inc(s_st, 16)

    bkeys = lambda b: [(b, g) for g in range(NZG)]
    store_cols(0, 0, 2400, bkeys(0) + [("ps1", 0)], "sync")
    store_cols(1, 0, 1920, bkeys(1), "scalar")
    store_cols(1, 1920, 2400, [("ps1", 1)], "sync")

    # ---- GPSIMD: wait for store completion (quiesce before the walrus
    # epilogue resets DMA state).  Sem clearing itself is left to the walrus
    # epilogue sweep, which zeroes all 256 semaphores every execution.
    nc.gpsimd.wait_ge(s_st, 3 * 16)

    nc.finalize()
    return nc


def _build_in_maps(coords: np.ndarray):
    import ml_dtypes

    coords = np.ascontiguousarray(coords, dtype=np.float32)
    assert coords.shape == (B * P, 3)
    f32 = np.float32

    xg = np.arange(W, dtype=f32)
    jcol = np.arange(-4, 6, dtype=f32)

    in_maps = []
    max_rows = 0
    for core in range(NCORES):
        z0 = core * DLOC
        zs = np.arange(z0, z0 + DLOC, dtype=f32)

        dx2 = np.full((128, W), 1.0e9, dtype=f32)
        tts = np.full((128, DLOC), -1.0e9, dtype=f32)
        oh = np.zeros((128, B * W), dtype=f32)

        r = 0
        for b in range(B):
            cb = coords[b * P : (b + 1) * P]
            for (pz, py, px) in cb:
                pz, py, px = f32(pz), f32(py), f32(px)
                dzt = (zs - pz).astype(f32)
                dzt = (dzt * dzt).astype(f32)
                yfl = f32(np.floor(py))
                ypr = (yfl + jcol).astype(f32)
                dyj = (ypr - py).astype(f32)
                dy2 = (dyj * dyj).astype(f32)
                cc = (f32(R2) - dy2).astype(f32)
                tloc = (cc[:, None] - dzt[None, :]).astype(f32)  # [J, DLOC]
                dxr = (xg - px).astype(f32)
                dxr2 = (dxr * dxr).astype(f32)
                for j in range(J):
                    y = ypr[j]
                    if y < 0 or y > H - 1:
                        continue
                    if tloc[j].max() <= 0.0:
                        continue
                    if r >= 128:
                        raise RuntimeError("kernel4: >128 active rows on a core")
                    dx2[r] = dxr2
                    tts[r] = tloc[j]
                    oh[r, b * W + int(y)] = 1.0
                    r += 1
        max_rows = max(max_rows, r)

        tab = np.concatenate([dx2, tts], axis=1)
        in_maps.append(
            {
                "tab": np.ascontiguousarray(tab, dtype=np.float32),
                "oh": np.ascontiguousarray(oh).astype(ml_dtypes.bfloat16),
            }
        )
    return max_rows, in_maps


def _get_program(npts=None):
    if "v3" not in _prog_cache:
        _prog_cache["v3"] = _build_program()
    return _prog_cache["v3"]


def kernel(x: np.ndarray, coords: np.ndarray) -> np.ndarray:
    from concourse.bass_utils import run_bass_kernel_spmd

    assert x.shape == (B, 4, D, H, W)
    _, in_maps = _build_in_maps(coords)
    nc = _get_program()
    res = run_bass_kernel_spmd(nc, in_maps, list(range(NCORES)))

    full = np.empty((B, 1, D, H, W), dtype=np.float32)
    for core in range(NCORES):
        zsl = slice(core * DLOC, (core + 1) * DLOC)
        oa = res.results[core]["outall"]  # [B, 128, 2400] u8
        om = oa[:, :, 0:1920].reshape(B, 128, DLOC, W).transpose(0, 2, 1, 3)
        full[:, 0, zsl, 0:128, :] = om
        o1 = oa[:, :, 1920:2400].reshape(B, NZG, 32, ZG, W)
        o1 = o1.transpose(0, 1, 3, 2, 4).reshape(B, DLOC, 32, W)
        full[:, 0, zsl, 128:160, :] = o1
    return full
